# revision 2
# baseline (speedup 1.0000x reference)
"""Trainium2 Bass kernel for NinjaTurtleProjLinear: y = x @ (mask*W)^T + b.

Full shapes: x (8192, 2048) f32, weight (8192, 2048) f32, bias (8192,) f32,
sparse_mask (8192, 2048) f32 -> y (8192, 8192) f32.

Strategy (column-parallel / tensor-parallel over out_features, 8 cores):
  - Host: transpose x once -> xt (2048, 8192), replicated to all cores.
    Shard weight/mask/bias along out_features; ship weight/mask shards
    pre-transposed (wt/mt: (2048, 1024)) so the device consumes them in
    matmul-ready layout. All arithmetic (mask multiply, matmul, bias add)
    runs on device.
  - Device (per core): wm = wt*mt on DVE (rounded to fp32r); stream xt in
    512-token windows; PE matmuls in fp32r (1 cycle/row) accumulating over
    the 2048-deep contraction in PSUM; DVE adds the bias during the
    PSUM->SBUF copy; DMA out the (8192, 1024) shard.
  - Host: concatenate the 8 output shards along out_features.
"""
import sys

sys.path.insert(0, "/opt/trn_rl_repo")

import numpy as np

N_TOK = 8192
IN_F = 2048
OUT_F = 8192
N_CORES = 8
OUTF_SH = OUT_F // N_CORES    # 1024
P = 128
K_T = IN_F // P               # 16 contraction tiles
TOK_WIN = 512                 # tokens per SBUF window
N_WIN = N_TOK // TOK_WIN      # 16
N_TT = TOK_WIN // P           # 4 token tiles per window
NB = 512                      # out_features per PSUM bank
N_NB = OUTF_SH // NB          # 2

_STATE = None


def _build_nc():
    import concourse.bass as bass
    import concourse.mybir as mybir
    import concourse.tile as tile
    from concourse import bacc

    f32 = mybir.dt.float32
    f32r = mybir.dt.float32r

    nc = bacc.Bacc(None)
    xt = nc.declare_dram_parameter("xt", [IN_F, N_TOK], f32r, isOutput=False)
    wt = nc.declare_dram_parameter("wt", [IN_F, OUTF_SH], f32, isOutput=False)
    mt = nc.declare_dram_parameter("mt", [IN_F, OUTF_SH], f32, isOutput=False)
    b = nc.declare_dram_parameter("b", [OUTF_SH], f32, isOutput=False)
    y = nc.declare_dram_parameter("y", [N_TOK, OUTF_SH], f32, isOutput=True)

    xt_r = xt[:].rearrange("(k p) t -> p k t", p=P)
    wt_r = wt[:].rearrange("(k p) n -> p k n", p=P)
    mt_r = mt[:].rearrange("(k p) n -> p k n", p=P)

    with tile.TileContext(nc) as tc:
        with (
            tc.tile_pool(name="const", bufs=1) as const_pool,
            tc.tile_pool(name="stage", bufs=2) as stage_pool,
            tc.tile_pool(name="xw", bufs=2) as xpool,
            tc.tile_pool(name="out", bufs=3) as opool,
            tc.tile_pool(name="ps", bufs=8, space="PSUM") as pspool,
        ):
            bias128 = const_pool.tile([P, OUTF_SH], f32)
            b_ap = b[:]
            b_bcast = bass.AP(tensor=b_ap.tensor, offset=b_ap.offset,
                              ap=[[0, P]] + list(b_ap.ap))
            nc.sync.dma_start(out=bias128[:], in_=b_bcast)

            wm = const_pool.tile([P, K_T, OUTF_SH], f32r)
            for k in range(K_T):
                wt_k = stage_pool.tile([P, OUTF_SH], f32, tag="wt")
                mt_k = stage_pool.tile([P, OUTF_SH], f32, tag="mt")
                nc.sync.dma_start(out=wt_k[:], in_=wt_r[:, k, :])
                nc.sync.dma_start(out=mt_k[:], in_=mt_r[:, k, :])
                nc.vector.tensor_mul(wm[:, k, :], wt_k[:], mt_k[:])

            for w in range(N_WIN):
                xwin = xpool.tile([P, K_T, TOK_WIN], f32r)
                nc.sync.dma_start(
                    out=xwin[:], in_=xt_r[:, :, w * TOK_WIN:(w + 1) * TOK_WIN])
                for tt in range(N_TT):
                    out_t = opool.tile([P, OUTF_SH], f32)
                    for nb in range(N_NB):
                        ps = pspool.tile([P, NB], f32)
                        for k in range(K_T):
                            nc.tensor.matmul(
                                ps[:],
                                xwin[:, k, tt * P:(tt + 1) * P],
                                wm[:, k, nb * NB:(nb + 1) * NB],
                                start=(k == 0),
                                stop=(k == K_T - 1),
                            )
                        nc.vector.tensor_add(
                            out_t[:, nb * NB:(nb + 1) * NB], ps[:],
                            bias128[:, nb * NB:(nb + 1) * NB])
                    t0 = w * TOK_WIN + tt * P
                    nc.sync.dma_start(out=y[t0:t0 + P, :], in_=out_t[:])
    nc.compile()
    return nc


def _make_runner():
    """Jitted 8-core SPMD executor (built once, reused across calls)."""
    import jax
    import numpy as _np
    from jax.sharding import Mesh, PartitionSpec
    from jax.experimental.shard_map import shard_map
    import concourse.mybir as mybir
    from concourse import bass2jax

    nc = _build_nc()
    bass2jax.install_neuronx_cc_hook()

    partition_name = (nc.partition_id_tensor.name
                      if nc.partition_id_tensor else None)
    in_names, out_names, out_avals = [], [], []
    for alloc in nc.m.functions[0].allocations:
        if not isinstance(alloc, mybir.MemoryLocationSet):
            continue
        name = alloc.memorylocations[0].name
        if alloc.kind == "ExternalInput":
            if name != partition_name:
                in_names.append(name)
        elif alloc.kind == "ExternalOutput":
            out_names.append(name)
            out_avals.append(jax.core.ShapedArray(
                tuple(alloc.tensor_shape), mybir.dt.np(alloc.dtype)))
    n_params = len(in_names)
    n_outs = len(out_names)
    all_in_names = in_names + out_names
    if partition_name is not None:
        all_in_names = all_in_names + [partition_name]

    def _body(*args):
        operands = list(args)
        if partition_name is not None:
            operands.append(bass2jax.partition_id_tensor())
        outs = bass2jax._bass_exec_p.bind(
            *operands,
            out_avals=tuple(out_avals),
            in_names=tuple(all_in_names),
            out_names=tuple(out_names),
            lowering_input_output_aliases=(),
            sim_require_finite=True,
            sim_require_nnan=True,
            nc=nc,
        )
        return tuple(outs)

    devices = jax.devices()[:N_CORES]
    mesh = Mesh(_np.asarray(devices), ("core",))
    in_specs = (PartitionSpec("core"),) * (n_params + n_outs)
    out_specs = (PartitionSpec("core"),) * n_outs
    donate = tuple(range(n_params, n_params + n_outs))
    sharded = jax.jit(
        shard_map(_body, mesh=mesh, in_specs=in_specs, out_specs=out_specs,
                  check_rep=False),
        donate_argnums=donate, keep_unused=True)

    out_shapes = [tuple(a.shape) for a in out_avals]
    out_dtypes = [a.dtype for a in out_avals]
    return sharded, in_names, out_names, out_shapes, out_dtypes


def kernel(x, weight, bias, sparse_mask):
    global _STATE
    x = np.asarray(x, dtype=np.float32)
    weight = np.asarray(weight, dtype=np.float32)
    bias = np.asarray(bias, dtype=np.float32)
    sparse_mask = np.asarray(sparse_mask, dtype=np.float32)

    if _STATE is None:
        _STATE = _make_runner()
    sharded, in_names, out_names, out_shapes, out_dtypes = _STATE

    xt = np.ascontiguousarray(x.T)
    per_core = []
    for c in range(N_CORES):
        sl = slice(c * OUTF_SH, (c + 1) * OUTF_SH)
        per_core.append({
            "xt": xt,
            "wt": np.ascontiguousarray(weight[sl].T),
            "mt": np.ascontiguousarray(sparse_mask[sl].T),
            "b": np.ascontiguousarray(bias[sl]),
        })

    concat_in = [np.concatenate([per_core[c][n] for c in range(N_CORES)], axis=0)
                 for n in in_names]
    concat_zeros = [np.zeros((N_CORES * s[0], *s[1:]), d)
                    for s, d in zip(out_shapes, out_dtypes)]
    out_arrs = sharded(*concat_in, *concat_zeros)

    yi = out_names.index("y")
    y_all = np.asarray(out_arrs[yi]).reshape(N_CORES, N_TOK, OUTF_SH)
    return np.concatenate(list(y_all), axis=1)


# revision 3
# speedup vs baseline: 1.0816x; 1.0816x over previous
"""Trainium2 Bass kernel for NinjaTurtleProjLinear: y = x @ (mask*W)^T + b.

Full shapes: x (8192, 2048) f32, weight (8192, 2048) f32, bias (8192,) f32,
sparse_mask (8192, 2048) f32 -> y (8192, 8192) f32.

Strategy (column-parallel / tensor-parallel over out_features, 8 cores):
  - Host: transpose x once -> xt (2048, 8192), replicated to all cores.
    Shard weight/mask/bias along out_features; ship weight/mask shards
    pre-transposed (wt/mt: (2048, 1024)) so the device consumes them in
    matmul-ready layout. Operands are shipped as fp16 (the mask is 0/1 so
    wm = wt*mt stays exact in fp16; PSUM accumulation is fp32).
    All arithmetic (mask multiply, matmul, bias add) runs on device.
  - Device (per core): wm = wt*mt on DVE; stream xt in 512-token windows;
    PE matmuls (fp16 in, fp32 accumulate, 1 cycle/row) over the 2048-deep
    contraction; DVE adds the bias during the PSUM->SBUF copy; DMA out the
    (8192, 1024) f32 shard. Input DMAs ride the SP HWDGE ring, output DMAs
    the ACT ring so loads never queue behind stores.
  - Host: concatenate the 8 output shards along out_features.
"""
import sys

sys.path.insert(0, "/opt/trn_rl_repo")

import numpy as np

N_TOK = 8192
IN_F = 2048
OUT_F = 8192
N_CORES = 8
OUTF_SH = OUT_F // N_CORES    # 1024
P = 128
K_T = IN_F // P               # 16 contraction tiles
TOK_WIN = 512                 # tokens per SBUF window
N_WIN = N_TOK // TOK_WIN      # 16
N_TT = TOK_WIN // P           # 4 token tiles per window
NB = 512                      # out_features per PSUM bank
N_NB = OUTF_SH // NB          # 2

USE_FP16 = True               # False -> fp32r operands (higher precision)

_STATE = None


def _build_nc():
    import concourse.bass as bass
    import concourse.mybir as mybir
    import concourse.tile as tile
    from concourse import bacc

    f32 = mybir.dt.float32
    op_dt = mybir.dt.float16 if USE_FP16 else mybir.dt.float32r

    nc = bacc.Bacc(None)
    xt = nc.declare_dram_parameter("xt", [IN_F, N_TOK], op_dt, isOutput=False)
    wt = nc.declare_dram_parameter("wt", [IN_F, OUTF_SH], op_dt, isOutput=False)
    mt = nc.declare_dram_parameter("mt", [IN_F, OUTF_SH], op_dt, isOutput=False)
    b = nc.declare_dram_parameter("b", [OUTF_SH], f32, isOutput=False)
    y = nc.declare_dram_parameter("y", [N_TOK, OUTF_SH], f32, isOutput=True)

    xt_r = xt[:].rearrange("(k p) t -> p k t", p=P)
    wt_r = wt[:].rearrange("(k p) n -> p k n", p=P)
    mt_r = mt[:].rearrange("(k p) n -> p k n", p=P)

    with tile.TileContext(nc) as tc:
        with (
            tc.tile_pool(name="const", bufs=1) as const_pool,
            tc.tile_pool(name="stage", bufs=2) as stage_pool,
            tc.tile_pool(name="xw", bufs=3) as xpool,
            tc.tile_pool(name="out", bufs=3) as opool,
            tc.tile_pool(name="ps", bufs=8, space="PSUM") as pspool,
        ):
            bias128 = const_pool.tile([P, OUTF_SH], f32)
            b_ap = b[:]
            b_bcast = bass.AP(tensor=b_ap.tensor, offset=b_ap.offset,
                              ap=[[0, P]] + list(b_ap.ap))
            nc.scalar.dma_start(out=bias128[:], in_=b_bcast)

            wm = const_pool.tile([P, K_T, OUTF_SH], op_dt)
            for k in range(K_T):
                wt_k = stage_pool.tile([P, OUTF_SH], op_dt, tag="wt")
                mt_k = stage_pool.tile([P, OUTF_SH], op_dt, tag="mt")
                nc.sync.dma_start(out=wt_k[:], in_=wt_r[:, k, :])
                nc.scalar.dma_start(out=mt_k[:], in_=mt_r[:, k, :])
                nc.vector.tensor_mul(wm[:, k, :], wt_k[:], mt_k[:])

            for w in range(N_WIN):
                xwin = xpool.tile([P, K_T, TOK_WIN], op_dt)
                nc.sync.dma_start(
                    out=xwin[:], in_=xt_r[:, :, w * TOK_WIN:(w + 1) * TOK_WIN])
                for tt in range(N_TT):
                    out_t = opool.tile([P, OUTF_SH], f32)
                    for nb in range(N_NB):
                        ps = pspool.tile([P, NB], f32)
                        for k in range(K_T):
                            nc.tensor.matmul(
                                ps[:],
                                xwin[:, k, tt * P:(tt + 1) * P],
                                wm[:, k, nb * NB:(nb + 1) * NB],
                                start=(k == 0),
                                stop=(k == K_T - 1),
                            )
                        nc.vector.tensor_add(
                            out_t[:, nb * NB:(nb + 1) * NB], ps[:],
                            bias128[:, nb * NB:(nb + 1) * NB])
                    t0 = w * TOK_WIN + tt * P
                    nc.scalar.dma_start(out=y[t0:t0 + P, :], in_=out_t[:])
    nc.compile()
    return nc


def _make_runner():
    """Jitted 8-core SPMD executor (built once, reused across calls)."""
    import jax
    import numpy as _np
    from jax.sharding import Mesh, PartitionSpec
    from jax.experimental.shard_map import shard_map
    import concourse.mybir as mybir
    from concourse import bass2jax

    nc = _build_nc()
    bass2jax.install_neuronx_cc_hook()

    partition_name = (nc.partition_id_tensor.name
                      if nc.partition_id_tensor else None)
    in_names, out_names, out_avals = [], [], []
    for alloc in nc.m.functions[0].allocations:
        if not isinstance(alloc, mybir.MemoryLocationSet):
            continue
        name = alloc.memorylocations[0].name
        if alloc.kind == "ExternalInput":
            if name != partition_name:
                in_names.append(name)
        elif alloc.kind == "ExternalOutput":
            out_names.append(name)
            out_avals.append(jax.core.ShapedArray(
                tuple(alloc.tensor_shape), mybir.dt.np(alloc.dtype)))
    n_params = len(in_names)
    n_outs = len(out_names)
    all_in_names = in_names + out_names
    if partition_name is not None:
        all_in_names = all_in_names + [partition_name]

    def _body(*args):
        operands = list(args)
        if partition_name is not None:
            operands.append(bass2jax.partition_id_tensor())
        outs = bass2jax._bass_exec_p.bind(
            *operands,
            out_avals=tuple(out_avals),
            in_names=tuple(all_in_names),
            out_names=tuple(out_names),
            lowering_input_output_aliases=(),
            sim_require_finite=True,
            sim_require_nnan=True,
            nc=nc,
        )
        return tuple(outs)

    devices = jax.devices()[:N_CORES]
    mesh = Mesh(_np.asarray(devices), ("core",))
    in_specs = (PartitionSpec("core"),) * (n_params + n_outs)
    out_specs = (PartitionSpec("core"),) * n_outs
    donate = tuple(range(n_params, n_params + n_outs))
    sharded = jax.jit(
        shard_map(_body, mesh=mesh, in_specs=in_specs, out_specs=out_specs,
                  check_rep=False),
        donate_argnums=donate, keep_unused=True)

    out_shapes = [tuple(a.shape) for a in out_avals]
    out_dtypes = [a.dtype for a in out_avals]
    return sharded, in_names, out_names, out_shapes, out_dtypes


def _per_core_inputs(x, weight, bias, sparse_mask):
    op_np = np.float16 if USE_FP16 else np.float32
    xt = np.ascontiguousarray(x.T.astype(op_np))
    per_core = []
    for c in range(N_CORES):
        sl = slice(c * OUTF_SH, (c + 1) * OUTF_SH)
        per_core.append({
            "xt": xt,
            "wt": np.ascontiguousarray(weight[sl].T.astype(op_np)),
            "mt": np.ascontiguousarray(sparse_mask[sl].T.astype(op_np)),
            "b": np.ascontiguousarray(bias[sl]),
        })
    return per_core


def kernel(x, weight, bias, sparse_mask):
    global _STATE
    x = np.asarray(x, dtype=np.float32)
    weight = np.asarray(weight, dtype=np.float32)
    bias = np.asarray(bias, dtype=np.float32)
    sparse_mask = np.asarray(sparse_mask, dtype=np.float32)

    if _STATE is None:
        _STATE = _make_runner()
    sharded, in_names, out_names, out_shapes, out_dtypes = _STATE

    per_core = _per_core_inputs(x, weight, bias, sparse_mask)
    concat_in = [np.concatenate([per_core[c][n] for c in range(N_CORES)], axis=0)
                 for n in in_names]
    concat_zeros = [np.zeros((N_CORES * s[0], *s[1:]), d)
                    for s, d in zip(out_shapes, out_dtypes)]
    out_arrs = sharded(*concat_in, *concat_zeros)

    yi = out_names.index("y")
    y_all = np.asarray(out_arrs[yi]).reshape(N_CORES, N_TOK, OUTF_SH)
    return np.concatenate(list(y_all), axis=1)


# revision 7
# speedup vs baseline: 1.1300x; 1.0448x over previous
"""Trainium2 Bass kernel for NinjaTurtleProjLinear: y = x @ (mask*W)^T + b.

Full shapes: x (8192, 2048) f32, weight (8192, 2048) f32, bias (8192,) f32,
sparse_mask (8192, 2048) f32 -> y (8192, 8192) f32.

Strategy (column-parallel / tensor-parallel over out_features, 8 cores):
  - Host: transpose x once -> xt (2048, 8192), replicated to all cores.
    Shard weight/mask/bias along out_features; ship weight/mask shards
    pre-transposed (wt/mt: (2048, 1024)) so the device consumes them in
    matmul-ready layout. Operands are shipped as fp16 (the mask is 0/1 so
    wm = wt*mt stays exact in fp16; PSUM accumulation is fp32).
    All arithmetic (mask multiply, matmul, bias add) runs on device.
  - Device (per core): wm = wt*mt on DVE; stream xt in 512-token windows;
    PE matmuls (fp16 in, fp32 accumulate, 1 cycle/row) over the 2048-deep
    contraction; DVE adds the bias during the PSUM->SBUF copy; DMA out the
    (8192, 1024) f32 shard. Input DMAs ride the SP HWDGE ring, output DMAs
    the ACT ring so loads never queue behind stores.
  - Host: concatenate the 8 output shards along out_features.
"""
import sys

sys.path.insert(0, "/opt/trn_rl_repo")

import numpy as np

N_TOK = 8192
IN_F = 2048
OUT_F = 8192
N_CORES = 8
OUTF_SH = OUT_F // N_CORES    # 1024
P = 128
K_T = IN_F // P               # 16 contraction tiles
TOK_WIN = 512                 # tokens per SBUF window
N_WIN = N_TOK // TOK_WIN      # 16
N_TT = TOK_WIN // P           # 4 token tiles per window
NB = 512                      # out_features per PSUM bank
N_NB = OUTF_SH // NB          # 2

USE_FP16 = True               # False -> fp32r operands (higher precision)

_STATE = None


def _build_nc():
    import concourse.bass as bass
    import concourse.mybir as mybir
    import concourse.tile as tile
    from concourse import bacc

    f32 = mybir.dt.float32
    op_dt = mybir.dt.float16 if USE_FP16 else mybir.dt.float32r

    nc = bacc.Bacc(None)
    xt = nc.declare_dram_parameter("xt", [IN_F, N_TOK], op_dt, isOutput=False)
    wt = nc.declare_dram_parameter("wt", [IN_F, OUTF_SH], op_dt, isOutput=False)
    mt = nc.declare_dram_parameter("mt", [IN_F, OUTF_SH], op_dt, isOutput=False)
    b = nc.declare_dram_parameter("b", [OUTF_SH], f32, isOutput=False)
    y = nc.declare_dram_parameter("y", [N_TOK, OUTF_SH], f32, isOutput=True)

    xt_r = xt[:].rearrange("(k p) t -> p k t", p=P)
    wt_r = wt[:].rearrange("(k p) n -> p k n", p=P)
    mt_r = mt[:].rearrange("(k p) n -> p k n", p=P)

    with tile.TileContext(nc) as tc:
        with (
            tc.tile_pool(name="const", bufs=1) as const_pool,
            tc.tile_pool(name="stage", bufs=2) as stage_pool,
            tc.tile_pool(name="xw", bufs=4) as xpool,
            tc.tile_pool(name="out", bufs=4) as opool,
            tc.tile_pool(name="ps", bufs=8, space="PSUM") as pspool,
        ):
            bias128 = const_pool.tile([P, OUTF_SH], f32)
            b_ap = b[:]
            b_bcast = bass.AP(tensor=b_ap.tensor, offset=b_ap.offset,
                              ap=[[0, P]] + list(b_ap.ap))
            nc.scalar.dma_start(out=bias128[:], in_=b_bcast)

            wm = const_pool.tile([P, K_T, OUTF_SH], op_dt)
            for k in range(K_T):
                wt_k = stage_pool.tile([P, OUTF_SH], op_dt, tag="wt")
                mt_k = stage_pool.tile([P, OUTF_SH], op_dt, tag="mt")
                nc.sync.dma_start(out=wt_k[:], in_=wt_r[:, k, :])
                nc.scalar.dma_start(out=mt_k[:], in_=mt_r[:, k, :])
                nc.vector.tensor_mul(wm[:, k, :], wt_k[:], mt_k[:])

            for w in range(N_WIN):
                xwin = xpool.tile([P, K_T, TOK_WIN], op_dt)
                nc.sync.dma_start(
                    out=xwin[:], in_=xt_r[:, :, w * TOK_WIN:(w + 1) * TOK_WIN])
                if w == 0:
                    # k-outer over all 8 PSUM banks: each weight k-tile is
                    # consumed by 8 matmuls the moment its DMA+mask-multiply
                    # lands, so the PE pipelines with the arriving wm tiles
                    # instead of idling until the full weight shard is in.
                    groups = [(tt, nb) for tt in range(N_TT)
                              for nb in range(N_NB)]
                    pss = [pspool.tile([P, NB], f32, tag="ps",
                                       name=f"ps_w0_{g}")
                           for g in range(len(groups))]
                    for k in range(K_T):
                        for g, (tt, nb) in enumerate(groups):
                            nc.tensor.matmul(
                                pss[g][:],
                                xwin[:, k, tt * P:(tt + 1) * P],
                                wm[:, k, nb * NB:(nb + 1) * NB],
                                start=(k == 0),
                                stop=(k == K_T - 1),
                            )
                    for g, (tt, nb) in enumerate(groups):
                        out_t = opool.tile([P, NB], f32, tag="out_h")
                        nc.vector.tensor_add(
                            out_t[:], pss[g][:],
                            bias128[:, nb * NB:(nb + 1) * NB])
                        t0 = w * TOK_WIN + tt * P
                        nc.scalar.dma_start(
                            out=y[t0:t0 + P, nb * NB:(nb + 1) * NB],
                            in_=out_t[:])
                    continue
                for tt in range(N_TT):
                    for nb in range(N_NB):
                        ps = pspool.tile([P, NB], f32)
                        for k in range(K_T):
                            nc.tensor.matmul(
                                ps[:],
                                xwin[:, k, tt * P:(tt + 1) * P],
                                wm[:, k, nb * NB:(nb + 1) * NB],
                                start=(k == 0),
                                stop=(k == K_T - 1),
                            )
                        out_t = opool.tile([P, NB], f32, tag="out_h")
                        nc.vector.tensor_add(
                            out_t[:], ps[:],
                            bias128[:, nb * NB:(nb + 1) * NB])
                        t0 = w * TOK_WIN + tt * P
                        nc.scalar.dma_start(
                            out=y[t0:t0 + P, nb * NB:(nb + 1) * NB],
                            in_=out_t[:])
    nc.compile()
    return nc


def _make_runner():
    """Jitted 8-core SPMD executor (built once, reused across calls)."""
    import jax
    import numpy as _np
    from jax.sharding import Mesh, PartitionSpec
    from jax.experimental.shard_map import shard_map
    import concourse.mybir as mybir
    from concourse import bass2jax

    nc = _build_nc()
    bass2jax.install_neuronx_cc_hook()

    partition_name = (nc.partition_id_tensor.name
                      if nc.partition_id_tensor else None)
    in_names, out_names, out_avals = [], [], []
    for alloc in nc.m.functions[0].allocations:
        if not isinstance(alloc, mybir.MemoryLocationSet):
            continue
        name = alloc.memorylocations[0].name
        if alloc.kind == "ExternalInput":
            if name != partition_name:
                in_names.append(name)
        elif alloc.kind == "ExternalOutput":
            out_names.append(name)
            out_avals.append(jax.core.ShapedArray(
                tuple(alloc.tensor_shape), mybir.dt.np(alloc.dtype)))
    n_params = len(in_names)
    n_outs = len(out_names)
    all_in_names = in_names + out_names
    if partition_name is not None:
        all_in_names = all_in_names + [partition_name]

    def _body(*args):
        operands = list(args)
        if partition_name is not None:
            operands.append(bass2jax.partition_id_tensor())
        outs = bass2jax._bass_exec_p.bind(
            *operands,
            out_avals=tuple(out_avals),
            in_names=tuple(all_in_names),
            out_names=tuple(out_names),
            lowering_input_output_aliases=(),
            sim_require_finite=True,
            sim_require_nnan=True,
            nc=nc,
        )
        return tuple(outs)

    devices = jax.devices()[:N_CORES]
    mesh = Mesh(_np.asarray(devices), ("core",))
    in_specs = (PartitionSpec("core"),) * (n_params + n_outs)
    out_specs = (PartitionSpec("core"),) * n_outs
    donate = tuple(range(n_params, n_params + n_outs))
    sharded = jax.jit(
        shard_map(_body, mesh=mesh, in_specs=in_specs, out_specs=out_specs,
                  check_rep=False),
        donate_argnums=donate, keep_unused=True)

    out_shapes = [tuple(a.shape) for a in out_avals]
    out_dtypes = [a.dtype for a in out_avals]
    return sharded, in_names, out_names, out_shapes, out_dtypes


def _per_core_inputs(x, weight, bias, sparse_mask):
    op_np = np.float16 if USE_FP16 else np.float32
    xt = np.ascontiguousarray(x.T.astype(op_np))
    per_core = []
    for c in range(N_CORES):
        sl = slice(c * OUTF_SH, (c + 1) * OUTF_SH)
        per_core.append({
            "xt": xt,
            "wt": np.ascontiguousarray(weight[sl].T.astype(op_np)),
            "mt": np.ascontiguousarray(sparse_mask[sl].T.astype(op_np)),
            "b": np.ascontiguousarray(bias[sl]),
        })
    return per_core


def kernel(x, weight, bias, sparse_mask):
    global _STATE
    x = np.asarray(x, dtype=np.float32)
    weight = np.asarray(weight, dtype=np.float32)
    bias = np.asarray(bias, dtype=np.float32)
    sparse_mask = np.asarray(sparse_mask, dtype=np.float32)

    if _STATE is None:
        _STATE = _make_runner()
    sharded, in_names, out_names, out_shapes, out_dtypes = _STATE

    per_core = _per_core_inputs(x, weight, bias, sparse_mask)
    concat_in = [np.concatenate([per_core[c][n] for c in range(N_CORES)], axis=0)
                 for n in in_names]
    concat_zeros = [np.zeros((N_CORES * s[0], *s[1:]), d)
                    for s, d in zip(out_shapes, out_dtypes)]
    out_arrs = sharded(*concat_in, *concat_zeros)

    yi = out_names.index("y")
    y_all = np.asarray(out_arrs[yi]).reshape(N_CORES, N_TOK, OUTF_SH)
    return np.concatenate(list(y_all), axis=1)


# revision 10
# speedup vs baseline: 2.5623x; 2.2675x over previous
"""Trainium2 Bass kernel for NinjaTurtleProjLinear: y = x @ (mask*W)^T + b.

Full shapes: x (8192, 2048) f32, weight (8192, 2048) f32, bias (8192,) f32,
sparse_mask (8192, 2048) f32 -> y (8192, 8192) f32.

Strategy (column-parallel over out_features, 8 cores, block-sparse):
  - The mask is banded: at [512 out x 128 in] block granularity most blocks
    are exactly zero, so (mask*W)^T block-columns that are all-zero are
    skipped — an exact optimization. The host analyzes the runtime mask
    into per-core nonzero-block schedules and builds one Bass program per
    core (each core only loads the x^T strips its blocks touch).
  - A fully-dense out-row (the gtoken row j=0) would force every k-strip
    onto one core; instead its 2048-deep GEMV is computed as width-1
    matmul groups distributed across the cores that already hold each
    k-strip, and the host sums the per-core partials into y[:, 0].
  - Operands ship as fp16 (mask is 0/1 so wm = wt*mt is exact in fp16);
    the mask multiply runs on DVE, matmuls accumulate fp32 in PSUM, DVE
    fuses the bias add into the PSUM->SBUF copy. Input DMAs ride the SP
    HWDGE ring, mask/bias/output DMAs the ACT ring.
"""
import sys

sys.path.insert(0, "/opt/trn_rl_repo")

import numpy as np

N_TOK = 8192
IN_F = 2048
OUT_F = 8192
N_CORES = 8
OUTF_SH = OUT_F // N_CORES    # 1024
P = 128
K_T = IN_F // P               # 16 k-strips of the full problem
TOK_WIN = 512                 # tokens per SBUF window
N_WIN = N_TOK // TOK_WIN      # 16
N_TT = TOK_WIN // P           # 4 token tiles per window
NB = 512                      # out_features per PSUM bank
N_NB = OUTF_SH // NB          # 2
N_TOKT = N_TOK // P           # 64 token tiles total

_STATE = None


def _analyze(sparse_mask):
    """Derive per-core block schedules from the runtime mask (exact skips)."""
    dense_row = bool(sparse_mask[0].sum() > IN_F // 2)
    pats = []
    for c in range(N_CORES):
        msh = np.asarray(sparse_mask[c * OUTF_SH:(c + 1) * OUTF_SH])
        mb = msh.copy()
        if c == 0 and dense_row:
            mb[0, :] = 0.0
        klists = []
        for nb in range(N_NB):
            blk = mb[nb * NB:(nb + 1) * NB]
            klists.append([kt for kt in range(K_T)
                           if blk[:, kt * P:(kt + 1) * P].any()])
        strips = sorted(set().union(*[set(kl) for kl in klists]))
        pats.append({"klists": klists, "strips": strips, "zero_row0": c == 0 and dense_row})
    if dense_row:
        # distribute the dense-row GEMV over cores that already hold each strip
        owner = {}
        for c in range(N_CORES):
            for k in pats[c]["strips"]:
                owner.setdefault(k, c)
        for k in range(K_T):
            if k not in owner:          # strip loaded by nobody: give to core 0
                owner[k] = 0
                pats[0]["strips"] = sorted(set(pats[0]["strips"]) | {k})
        for c in range(N_CORES):
            pats[c]["y0k"] = sorted(k for k, oc in owner.items() if oc == c)
    else:
        for c in range(N_CORES):
            pats[c]["y0k"] = []
    return pats, dense_row


def _build_nc_core(pat):
    import concourse.bass as bass
    import concourse.mybir as mybir
    import concourse.tile as tile
    from concourse import bacc

    f32 = mybir.dt.float32
    f16 = mybir.dt.float16

    strips = pat["strips"]
    S = len(strips)
    slot = {k: i for i, k in enumerate(strips)}
    klists = [[slot[k] for k in kl] for kl in pat["klists"]]
    y0s = [slot[k] for k in pat["y0k"]]
    n0 = len(y0s)

    nc = bacc.Bacc(None)
    xt = nc.declare_dram_parameter("xt", [S * P, N_TOK], f16, isOutput=False)
    wt = nc.declare_dram_parameter("wt", [S * P, OUTF_SH], f16, isOutput=False)
    mt = nc.declare_dram_parameter("mt", [S * P, OUTF_SH], f16, isOutput=False)
    b = nc.declare_dram_parameter("b", [OUTF_SH], f32, isOutput=False)
    y = nc.declare_dram_parameter("y", [N_TOK, OUTF_SH], f32, isOutput=True)
    if n0:
        w0 = nc.declare_dram_parameter("w0", [n0 * P], f16, isOutput=False)
        y0p = nc.declare_dram_parameter("y0p", [P, N_TOKT], f32, isOutput=True)

    xt_r = xt[:].rearrange("(s p) t -> p s t", p=P)
    wt_r = wt[:].rearrange("(s p) n -> p s n", p=P)
    mt_r = mt[:].rearrange("(s p) n -> p s n", p=P)

    with tile.TileContext(nc) as tc:
        with (
            tc.tile_pool(name="const", bufs=1) as const_pool,
            tc.tile_pool(name="stage", bufs=2) as stage_pool,
            tc.tile_pool(name="xw", bufs=4) as xpool,
            tc.tile_pool(name="out", bufs=4) as opool,
            tc.tile_pool(name="ps", bufs=6, space="PSUM") as pspool,
            tc.tile_pool(name="ps1", bufs=2, space="PSUM") as ps1pool,
        ):
            bias128 = const_pool.tile([P, OUTF_SH], f32)
            b_ap = b[:]
            b_bcast = bass.AP(tensor=b_ap.tensor, offset=b_ap.offset,
                              ap=[[0, P]] + list(b_ap.ap))
            nc.scalar.dma_start(out=bias128[:], in_=b_bcast)

            wm = const_pool.tile([P, S, OUTF_SH], f16)
            for s in range(S):
                wt_s = stage_pool.tile([P, OUTF_SH], f16, tag="wt")
                mt_s = stage_pool.tile([P, OUTF_SH], f16, tag="mt")
                nc.sync.dma_start(out=wt_s[:], in_=wt_r[:, s, :])
                nc.scalar.dma_start(out=mt_s[:], in_=mt_r[:, s, :])
                nc.vector.tensor_mul(wm[:, s, :], wt_s[:], mt_s[:])

            if n0:
                w0_sb = const_pool.tile([P, n0], f16)
                nc.sync.dma_start(
                    out=w0_sb[:], in_=w0[:].rearrange("(i p) -> p i", p=P))
                y0_sb = const_pool.tile([P, N_TOKT], f32)

            def mm_group(ps_t, xwin, tt, slots, nslice):
                for j, s in enumerate(slots):
                    nc.tensor.matmul(
                        ps_t[:],
                        xwin[:, s, tt * P:(tt + 1) * P],
                        wm[:, s, nslice],
                        start=(j == 0),
                        stop=(j == len(slots) - 1),
                    )

            def drain(ps_t, nb, t0):
                out_t = opool.tile([P, NB], f32, tag="out_h",
                                   name=f"out_{t0}_{nb}")
                nc.vector.tensor_add(
                    out_t[:], ps_t[:], bias128[:, nb * NB:(nb + 1) * NB])
                nc.scalar.dma_start(
                    out=y[t0:t0 + P, nb * NB:(nb + 1) * NB], in_=out_t[:])

            def y0_group(xwin, tt, ti):
                ps0 = ps1pool.tile([P, 1], f32, tag="ps0", name=f"ps0_{ti}")
                for j, s in enumerate(y0s):
                    si = y0s.index(s)
                    nc.tensor.matmul(
                        ps0[:],
                        xwin[:, s, tt * P:(tt + 1) * P],
                        w0_sb[:, si:si + 1],
                        start=(j == 0),
                        stop=(j == n0 - 1),
                    )
                nc.vector.tensor_copy(y0_sb[:, ti:ti + 1], ps0[:])

            for w in range(N_WIN):
                xwin = xpool.tile([P, S, TOK_WIN], f16)
                nc.sync.dma_start(
                    out=xwin[:], in_=xt_r[:, :, w * TOK_WIN:(w + 1) * TOK_WIN])
                if w == 0:
                    # k-outer in chunks of 4 groups: matmuls start as soon as
                    # each weight strip's DMA + mask-multiply lands.
                    groups = [(tt, nb) for tt in range(N_TT)
                              for nb in range(N_NB) if klists[nb]]
                    for chunk in range(0, len(groups), 4):
                        sub = groups[chunk:chunk + 4]
                        pss = [pspool.tile([P, NB], f32, tag="ps",
                                           name=f"ps_w0_{chunk}_{g}")
                               for g in range(len(sub))]
                        for s in range(S):
                            for g, (tt, nb) in enumerate(sub):
                                kl = klists[nb]
                                if s not in kl:
                                    continue
                                j = kl.index(s)
                                nc.tensor.matmul(
                                    pss[g][:],
                                    xwin[:, s, tt * P:(tt + 1) * P],
                                    wm[:, s, nb * NB:(nb + 1) * NB],
                                    start=(j == 0),
                                    stop=(j == len(kl) - 1),
                                )
                        for g, (tt, nb) in enumerate(sub):
                            drain(pss[g], nb, tt * P)
                    for tt in range(N_TT):
                        for nb in range(N_NB):
                            if not klists[nb]:
                                t0 = tt * P
                                out_t = opool.tile([P, NB], f32, tag="out_h",
                                                   name=f"outz_{tt}_{nb}")
                                nc.vector.tensor_copy(
                                    out_t[:], bias128[:, nb * NB:(nb + 1) * NB])
                                nc.scalar.dma_start(
                                    out=y[t0:t0 + P, nb * NB:(nb + 1) * NB],
                                    in_=out_t[:])
                        if n0:
                            y0_group(xwin, tt, tt)
                    continue
                for tt in range(N_TT):
                    t0 = w * TOK_WIN + tt * P
                    for nb in range(N_NB):
                        kl = klists[nb]
                        if not kl:
                            out_t = opool.tile([P, NB], f32, tag="out_h",
                                               name=f"outz_{w}_{tt}_{nb}")
                            nc.vector.tensor_copy(
                                out_t[:], bias128[:, nb * NB:(nb + 1) * NB])
                            nc.scalar.dma_start(
                                out=y[t0:t0 + P, nb * NB:(nb + 1) * NB],
                                in_=out_t[:])
                            continue
                        ps = pspool.tile([P, NB], f32, tag="ps",
                                         name=f"ps_{w}_{tt}_{nb}")
                        mm_group(ps, xwin, tt, kl, slice(nb * NB, (nb + 1) * NB))
                        drain(ps, nb, t0)
                    if n0:
                        y0_group(xwin, tt, w * N_TT + tt)
            if n0:
                nc.scalar.dma_start(out=y0p[:], in_=y0_sb[:])
    nc.compile()
    return nc


def _make_core_runner(nc):
    import jax
    import concourse.mybir as mybir
    from concourse import bass2jax

    partition_name = (nc.partition_id_tensor.name
                      if nc.partition_id_tensor else None)
    in_names, out_names, out_avals = [], [], []
    for alloc in nc.m.functions[0].allocations:
        if not isinstance(alloc, mybir.MemoryLocationSet):
            continue
        name = alloc.memorylocations[0].name
        if alloc.kind == "ExternalInput":
            if name != partition_name:
                in_names.append(name)
        elif alloc.kind == "ExternalOutput":
            out_names.append(name)
            out_avals.append(jax.core.ShapedArray(
                tuple(alloc.tensor_shape), mybir.dt.np(alloc.dtype)))
    n_params = len(in_names)
    n_outs = len(out_names)
    all_in_names = list(in_names) + list(out_names)
    if partition_name is not None:
        all_in_names = all_in_names + [partition_name]

    def _body(*args):
        operands = list(args)
        if partition_name is not None:
            operands.append(bass2jax.partition_id_tensor())
        outs = bass2jax._bass_exec_p.bind(
            *operands,
            out_avals=tuple(out_avals),
            in_names=tuple(all_in_names),
            out_names=tuple(out_names),
            lowering_input_output_aliases=(),
            sim_require_finite=True,
            sim_require_nnan=True,
            nc=nc,
        )
        return tuple(outs)

    donate = tuple(range(n_params, n_params + n_outs))
    fn = jax.jit(_body, donate_argnums=donate, keep_unused=True)
    out_shapes = [tuple(a.shape) for a in out_avals]
    out_dtypes = [a.dtype for a in out_avals]
    return fn, in_names, out_names, out_shapes, out_dtypes


def _pack_inputs(pat, c, xt16, weight, bias, sparse_mask):
    strips = pat["strips"]
    sl = slice(c * OUTF_SH, (c + 1) * OUTF_SH)
    xt_p = np.concatenate([xt16[k * P:(k + 1) * P] for k in strips], axis=0)
    wsh_t = weight[sl].T  # (IN_F, OUTF_SH)
    msh_t = np.asarray(sparse_mask[sl]).T.copy()
    if pat["zero_row0"]:
        msh_t[:, 0] = 0.0
    wt_p = np.concatenate(
        [wsh_t[k * P:(k + 1) * P] for k in strips], axis=0).astype(np.float16)
    mt_p = np.concatenate(
        [msh_t[k * P:(k + 1) * P] for k in strips], axis=0).astype(np.float16)
    ins = {"xt": np.ascontiguousarray(xt_p),
           "wt": np.ascontiguousarray(wt_p),
           "mt": np.ascontiguousarray(mt_p),
           "b": np.ascontiguousarray(bias[sl])}
    if pat["y0k"]:
        w0_full = (weight[0] * np.asarray(sparse_mask[0])).astype(np.float16)
        ins["w0"] = np.ascontiguousarray(np.concatenate(
            [w0_full[k * P:(k + 1) * P] for k in pat["y0k"]]))
    return ins


def _make_runner(sparse_mask):
    import jax

    pats, dense_row = _analyze(sparse_mask)
    runners = []
    for c in range(N_CORES):
        nc = _build_nc_core(pats[c])
        runners.append(_make_core_runner(nc))
    devices = jax.devices()[:N_CORES]
    return pats, dense_row, runners, devices


def kernel(x, weight, bias, sparse_mask):
    global _STATE
    import jax

    x = np.asarray(x, dtype=np.float32)
    weight = np.asarray(weight, dtype=np.float32)
    bias = np.asarray(bias, dtype=np.float32)
    sparse_mask = np.asarray(sparse_mask, dtype=np.float32)

    mask_key = hash(sparse_mask.tobytes())
    if _STATE is None or _STATE[0] != mask_key:
        _STATE = (mask_key, _make_runner(sparse_mask))
    _, (pats, dense_row, runners, devices) = _STATE

    xt16 = np.ascontiguousarray(x.T.astype(np.float16))
    futures = []
    for c in range(N_CORES):
        fn, in_names, out_names, out_shapes, out_dtypes = runners[c]
        ins = _pack_inputs(pats[c], c, xt16, weight, bias, sparse_mask)
        args = [jax.device_put(ins[n], devices[c]) for n in in_names]
        zeros = [jax.device_put(np.zeros(s, d), devices[c])
                 for s, d in zip(out_shapes, out_dtypes)]
        futures.append(fn(*args, *zeros))

    y_parts = []
    y0_sum = None
    for c in range(N_CORES):
        _, _, out_names, _, _ = runners[c]
        outs = futures[c]
        om = {n: outs[i] for i, n in enumerate(out_names)}
        y_parts.append(np.asarray(om["y"]))
        if "y0p" in om:
            p = np.asarray(om["y0p"])  # [P, N_TOKT], element [p, t] = tok t*P+p
            y0_sum = p if y0_sum is None else y0_sum + p
    y_full = np.concatenate(y_parts, axis=1)
    if dense_row and y0_sum is not None:
        y_full[:, 0] = y0_sum.T.reshape(N_TOK) + bias[0]
    return y_full


# revision 11
# speedup vs baseline: 3.0480x; 1.1896x over previous
"""Trainium2 Bass kernel for NinjaTurtleProjLinear: y = x @ (mask*W)^T + b.

Full shapes: x (8192, 2048) f32, weight (8192, 2048) f32, bias (8192,) f32,
sparse_mask (8192, 2048) f32 -> y (8192, 8192) f32.

Strategy (column-parallel over out_features, 8 cores, block-sparse):
  - The mask is banded: at [512 out x 128 in] block granularity most blocks
    are exactly zero, so (mask*W)^T block-columns that are all-zero are
    skipped — an exact optimization. The host analyzes the runtime mask
    into per-core nonzero-block schedules and builds one Bass program per
    core (each core only loads the x^T strips its blocks touch).
  - A fully-dense out-row (the gtoken row j=0) would force every k-strip
    onto one core; instead its 2048-deep GEMV is computed as width-1
    matmul groups distributed across the cores that already hold each
    k-strip, and the host sums the per-core partials into y[:, 0].
  - Operands ship as fp16 (mask is 0/1 so wm = wt*mt is exact in fp16);
    the mask multiply runs on DVE, matmuls accumulate fp32 in PSUM, DVE
    fuses the bias add into the PSUM->SBUF copy. Input DMAs ride the SP
    HWDGE ring, mask/bias/output DMAs the ACT ring.
"""
import sys

sys.path.insert(0, "/opt/trn_rl_repo")

import numpy as np

N_TOK = 8192
IN_F = 2048
OUT_F = 8192
N_CORES = 8
OUTF_SH = OUT_F // N_CORES    # 1024
P = 128
K_T = IN_F // P               # 16 k-strips of the full problem
TOK_WIN = 512                 # tokens per SBUF window
N_WIN = N_TOK // TOK_WIN      # 16
N_TT = TOK_WIN // P           # 4 token tiles per window
NB = 256                      # out_features per PSUM block (finer = better skip)
N_NB = OUTF_SH // NB          # 4
N_TOKT = N_TOK // P           # 64 token tiles total

_STATE = None


def _analyze(sparse_mask):
    """Derive per-core block schedules from the runtime mask (exact skips)."""
    dense_row = bool(sparse_mask[0].sum() > IN_F // 2)
    pats = []
    for c in range(N_CORES):
        msh = np.asarray(sparse_mask[c * OUTF_SH:(c + 1) * OUTF_SH])
        mb = msh.copy()
        if c == 0 and dense_row:
            mb[0, :] = 0.0
        klists = []
        for nb in range(N_NB):
            blk = mb[nb * NB:(nb + 1) * NB]
            klists.append([kt for kt in range(K_T)
                           if blk[:, kt * P:(kt + 1) * P].any()])
        strips = sorted(set().union(*[set(kl) for kl in klists]))
        pats.append({"klists": klists, "strips": strips, "zero_row0": c == 0 and dense_row})
    if dense_row:
        # distribute the dense-row GEMV over cores that already hold each strip
        owner = {}
        for c in range(N_CORES):
            for k in pats[c]["strips"]:
                owner.setdefault(k, c)
        for k in range(K_T):
            if k not in owner:          # strip loaded by nobody: give to core 0
                owner[k] = 0
                pats[0]["strips"] = sorted(set(pats[0]["strips"]) | {k})
        for c in range(N_CORES):
            pats[c]["y0k"] = sorted(k for k, oc in owner.items() if oc == c)
    else:
        for c in range(N_CORES):
            pats[c]["y0k"] = []
    return pats, dense_row


def _build_nc_core(pat):
    import concourse.bass as bass
    import concourse.mybir as mybir
    import concourse.tile as tile
    from concourse import bacc

    f32 = mybir.dt.float32
    f16 = mybir.dt.float16

    strips = pat["strips"]
    S = len(strips)
    slot = {k: i for i, k in enumerate(strips)}
    klists = [[slot[k] for k in kl] for kl in pat["klists"]]
    y0s = [slot[k] for k in pat["y0k"]]
    n0 = len(y0s)

    nc = bacc.Bacc(None)
    xt = nc.declare_dram_parameter("xt", [S * P, N_TOK], f16, isOutput=False)
    wt = nc.declare_dram_parameter("wt", [S * P, OUTF_SH], f16, isOutput=False)
    mt = nc.declare_dram_parameter("mt", [S * P, OUTF_SH], f16, isOutput=False)
    b = nc.declare_dram_parameter("b", [OUTF_SH], f32, isOutput=False)
    y = nc.declare_dram_parameter("y", [N_TOK, OUTF_SH], f32, isOutput=True)
    if n0:
        w0 = nc.declare_dram_parameter("w0", [n0 * P], f16, isOutput=False)
        y0p = nc.declare_dram_parameter("y0p", [P, N_TOKT], f32, isOutput=True)

    xt_r = xt[:].rearrange("(s p) t -> p s t", p=P)
    wt_r = wt[:].rearrange("(s p) n -> p s n", p=P)
    mt_r = mt[:].rearrange("(s p) n -> p s n", p=P)

    with tile.TileContext(nc) as tc:
        with (
            tc.tile_pool(name="const", bufs=1) as const_pool,
            tc.tile_pool(name="stage", bufs=2) as stage_pool,
            tc.tile_pool(name="xw", bufs=4) as xpool,
            tc.tile_pool(name="out", bufs=4) as opool,
            tc.tile_pool(name="ps", bufs=6, space="PSUM") as pspool,
            tc.tile_pool(name="ps1", bufs=2, space="PSUM") as ps1pool,
        ):
            bias128 = const_pool.tile([P, OUTF_SH], f32)
            b_ap = b[:]
            b_bcast = bass.AP(tensor=b_ap.tensor, offset=b_ap.offset,
                              ap=[[0, P]] + list(b_ap.ap))
            nc.scalar.dma_start(out=bias128[:], in_=b_bcast)

            wm = const_pool.tile([P, S, OUTF_SH], f16)
            for s in range(S):
                wt_s = stage_pool.tile([P, OUTF_SH], f16, tag="wt")
                mt_s = stage_pool.tile([P, OUTF_SH], f16, tag="mt")
                nc.sync.dma_start(out=wt_s[:], in_=wt_r[:, s, :])
                nc.scalar.dma_start(out=mt_s[:], in_=mt_r[:, s, :])
                nc.vector.tensor_mul(wm[:, s, :], wt_s[:], mt_s[:])

            if n0:
                w0_sb = const_pool.tile([P, n0], f16)
                nc.sync.dma_start(
                    out=w0_sb[:], in_=w0[:].rearrange("(i p) -> p i", p=P))
                y0_sb = const_pool.tile([P, N_TOKT], f32)

            def mm_group(ps_t, xwin, tt, slots, nslice):
                for j, s in enumerate(slots):
                    nc.tensor.matmul(
                        ps_t[:],
                        xwin[:, s, tt * P:(tt + 1) * P],
                        wm[:, s, nslice],
                        start=(j == 0),
                        stop=(j == len(slots) - 1),
                    )

            def drain(ps_t, nb, out_t):
                nc.vector.tensor_add(
                    out_t[:, nb * NB:(nb + 1) * NB], ps_t[:],
                    bias128[:, nb * NB:(nb + 1) * NB])

            def y0_group(xwin, tt, ti):
                ps0 = ps1pool.tile([P, 1], f32, tag="ps0", name=f"ps0_{ti}")
                for j, s in enumerate(y0s):
                    si = y0s.index(s)
                    nc.tensor.matmul(
                        ps0[:],
                        xwin[:, s, tt * P:(tt + 1) * P],
                        w0_sb[:, si:si + 1],
                        start=(j == 0),
                        stop=(j == n0 - 1),
                    )
                nc.vector.tensor_copy(y0_sb[:, ti:ti + 1], ps0[:])

            for w in range(N_WIN):
                xwin = xpool.tile([P, S, TOK_WIN], f16)
                nc.sync.dma_start(
                    out=xwin[:], in_=xt_r[:, :, w * TOK_WIN:(w + 1) * TOK_WIN])
                if w == 0:
                    # k-outer, one token tile (up to 4 psum groups) per chunk:
                    # matmuls start as soon as each weight strip's DMA +
                    # mask-multiply lands.
                    for tt in range(N_TT):
                        out_t = opool.tile([P, OUTF_SH], f32, tag="out_h",
                                           name=f"out_w0_{tt}")
                        sub = [nb for nb in range(N_NB) if klists[nb]]
                        pss = [pspool.tile([P, NB], f32, tag="ps",
                                           name=f"ps_w0_{tt}_{g}")
                               for g in range(len(sub))]
                        for s in range(S):
                            for g, nb in enumerate(sub):
                                kl = klists[nb]
                                if s not in kl:
                                    continue
                                j = kl.index(s)
                                nc.tensor.matmul(
                                    pss[g][:],
                                    xwin[:, s, tt * P:(tt + 1) * P],
                                    wm[:, s, nb * NB:(nb + 1) * NB],
                                    start=(j == 0),
                                    stop=(j == len(kl) - 1),
                                )
                        for g, nb in enumerate(sub):
                            drain(pss[g], nb, out_t)
                        for nb in range(N_NB):
                            if not klists[nb]:
                                nc.vector.tensor_copy(
                                    out_t[:, nb * NB:(nb + 1) * NB],
                                    bias128[:, nb * NB:(nb + 1) * NB])
                        nc.scalar.dma_start(out=y[tt * P:(tt + 1) * P, :],
                                            in_=out_t[:])
                        if n0:
                            y0_group(xwin, tt, tt)
                    continue
                for tt in range(N_TT):
                    t0 = w * TOK_WIN + tt * P
                    out_t = opool.tile([P, OUTF_SH], f32, tag="out_h",
                                       name=f"out_{w}_{tt}")
                    for nb in range(N_NB):
                        kl = klists[nb]
                        if not kl:
                            nc.vector.tensor_copy(
                                out_t[:, nb * NB:(nb + 1) * NB],
                                bias128[:, nb * NB:(nb + 1) * NB])
                            continue
                        ps = pspool.tile([P, NB], f32, tag="ps",
                                         name=f"ps_{w}_{tt}_{nb}")
                        mm_group(ps, xwin, tt, kl, slice(nb * NB, (nb + 1) * NB))
                        drain(ps, nb, out_t)
                    nc.scalar.dma_start(out=y[t0:t0 + P, :], in_=out_t[:])
                    if n0:
                        y0_group(xwin, tt, w * N_TT + tt)
            if n0:
                nc.scalar.dma_start(out=y0p[:], in_=y0_sb[:])
    nc.compile()
    return nc


def _make_core_runner(nc):
    import jax
    import concourse.mybir as mybir
    from concourse import bass2jax

    partition_name = (nc.partition_id_tensor.name
                      if nc.partition_id_tensor else None)
    in_names, out_names, out_avals = [], [], []
    for alloc in nc.m.functions[0].allocations:
        if not isinstance(alloc, mybir.MemoryLocationSet):
            continue
        name = alloc.memorylocations[0].name
        if alloc.kind == "ExternalInput":
            if name != partition_name:
                in_names.append(name)
        elif alloc.kind == "ExternalOutput":
            out_names.append(name)
            out_avals.append(jax.core.ShapedArray(
                tuple(alloc.tensor_shape), mybir.dt.np(alloc.dtype)))
    n_params = len(in_names)
    n_outs = len(out_names)
    all_in_names = list(in_names) + list(out_names)
    if partition_name is not None:
        all_in_names = all_in_names + [partition_name]

    def _body(*args):
        operands = list(args)
        if partition_name is not None:
            operands.append(bass2jax.partition_id_tensor())
        outs = bass2jax._bass_exec_p.bind(
            *operands,
            out_avals=tuple(out_avals),
            in_names=tuple(all_in_names),
            out_names=tuple(out_names),
            lowering_input_output_aliases=(),
            sim_require_finite=True,
            sim_require_nnan=True,
            nc=nc,
        )
        return tuple(outs)

    donate = tuple(range(n_params, n_params + n_outs))
    fn = jax.jit(_body, donate_argnums=donate, keep_unused=True)
    out_shapes = [tuple(a.shape) for a in out_avals]
    out_dtypes = [a.dtype for a in out_avals]
    return fn, in_names, out_names, out_shapes, out_dtypes


def _pack_inputs(pat, c, xt16, weight, bias, sparse_mask):
    strips = pat["strips"]
    sl = slice(c * OUTF_SH, (c + 1) * OUTF_SH)
    xt_p = np.concatenate([xt16[k * P:(k + 1) * P] for k in strips], axis=0)
    wsh_t = weight[sl].T  # (IN_F, OUTF_SH)
    msh_t = np.asarray(sparse_mask[sl]).T.copy()
    if pat["zero_row0"]:
        msh_t[:, 0] = 0.0
    wt_p = np.concatenate(
        [wsh_t[k * P:(k + 1) * P] for k in strips], axis=0).astype(np.float16)
    mt_p = np.concatenate(
        [msh_t[k * P:(k + 1) * P] for k in strips], axis=0).astype(np.float16)
    ins = {"xt": np.ascontiguousarray(xt_p),
           "wt": np.ascontiguousarray(wt_p),
           "mt": np.ascontiguousarray(mt_p),
           "b": np.ascontiguousarray(bias[sl])}
    if pat["y0k"]:
        w0_full = (weight[0] * np.asarray(sparse_mask[0])).astype(np.float16)
        ins["w0"] = np.ascontiguousarray(np.concatenate(
            [w0_full[k * P:(k + 1) * P] for k in pat["y0k"]]))
    return ins


def _make_runner(sparse_mask):
    import jax

    pats, dense_row = _analyze(sparse_mask)
    runners = []
    for c in range(N_CORES):
        nc = _build_nc_core(pats[c])
        runners.append(_make_core_runner(nc))
    devices = jax.devices()[:N_CORES]
    return pats, dense_row, runners, devices


def kernel(x, weight, bias, sparse_mask):
    global _STATE
    import jax

    x = np.asarray(x, dtype=np.float32)
    weight = np.asarray(weight, dtype=np.float32)
    bias = np.asarray(bias, dtype=np.float32)
    sparse_mask = np.asarray(sparse_mask, dtype=np.float32)

    mask_key = hash(sparse_mask.tobytes())
    if _STATE is None or _STATE[0] != mask_key:
        _STATE = (mask_key, _make_runner(sparse_mask))
    _, (pats, dense_row, runners, devices) = _STATE

    xt16 = np.ascontiguousarray(x.T.astype(np.float16))
    futures = []
    for c in range(N_CORES):
        fn, in_names, out_names, out_shapes, out_dtypes = runners[c]
        ins = _pack_inputs(pats[c], c, xt16, weight, bias, sparse_mask)
        args = [jax.device_put(ins[n], devices[c]) for n in in_names]
        zeros = [jax.device_put(np.zeros(s, d), devices[c])
                 for s, d in zip(out_shapes, out_dtypes)]
        futures.append(fn(*args, *zeros))

    y_parts = []
    y0_sum = None
    for c in range(N_CORES):
        _, _, out_names, _, _ = runners[c]
        outs = futures[c]
        om = {n: outs[i] for i, n in enumerate(out_names)}
        y_parts.append(np.asarray(om["y"]))
        if "y0p" in om:
            p = np.asarray(om["y0p"])  # [P, N_TOKT], element [p, t] = tok t*P+p
            y0_sum = p if y0_sum is None else y0_sum + p
    y_full = np.concatenate(y_parts, axis=1)
    if dense_row and y0_sum is not None:
        y_full[:, 0] = y0_sum.T.reshape(N_TOK) + bias[0]
    return y_full
